# revision 6
# baseline (speedup 1.0000x reference)
"""Trainium2 Bass kernel for the "no two consecutive > threshold" recurrence.

Reference semantics (per row, scanning along the seq axis S):
    out[0] = x[0]
    out[t] = x[t] * (1 - (out[t-1] > 0.5) * (x[t] > 0.5))

Key transformation: with d0[t] = 0.5 + (x[t] <= 0.5)  (i.e. 1.5 for small x,
0.5 for large x), the recurrence is exactly

    out[t] = x[t] * (d0[t] >= out[t-1])

because out[t-1] < 1.0 always (so d0 = 1.5 always passes), and d0 = 0.5
implements the (out[t-1] > 0.5) kill test. This maps 1:1 onto the DVE
``tensor_tensor_scan`` instruction:

    state = (data0[:,t] op0 state) op1 data1[:,t]
          = (d0[:,t] is_ge state) mult x[:,t]

so the whole kernel is, per [128, S] tile: one fused tensor_scalar (on
GPSIMD) to build d0, one tensor_tensor_scan (on DVE) that directly produces
the final output, and the two DMAs. Memory-bound by design.

Sharding: embarrassingly data-parallel over the batch axis -- 4096 rows
split as 8 x 512 contiguous row blocks, one per NeuronCore.
"""

import numpy as np

_B, _S = 4096, 8192  # full input shape [B, S] float32
_NC = 8  # NeuronCores
_RPC = _B // _NC  # rows per core = 512
_P = 128  # SBUF partitions
_NT = _RPC // _P  # row tiles per core = 4

_cache = {}

# Tunables (chosen via TimelineSim sweeps: chunks=2/bufs=4 hits the DMA-only
# floor of 96.6us; chunks=1 pays ~11us of pipeline fill/drain).
_CHUNKS = 2  # seq chunks per [128, S] row tile
_XBUFS = 4
_DBUFS = 4


def _build(chunks=_CHUNKS, xbufs=_XBUFS, dbufs=_DBUFS, repeat=1):
    import concourse.bacc as bacc
    import concourse.mybir as mybir
    from concourse.tile import TileContext

    Alu = mybir.AluOpType
    f32 = mybir.dt.float32
    cw = _S // chunks  # chunk width along seq

    nc = bacc.Bacc("TRN2", debug=False, num_devices=_NC)
    x_d = nc.dram_tensor("x", (_RPC, _S), f32, kind="ExternalInput").ap()
    y_d = nc.dram_tensor("y", (_RPC, _S), f32, kind="ExternalOutput").ap()

    with TileContext(nc) as tc:
        with tc.tile_pool(name="sbuf", bufs=2) as pool:
            for rep in range(repeat):
                for i in range(_NT):
                    r0, r1 = i * _P, (i + 1) * _P
                    prev = None  # previous chunk's output tile (for scan carry)
                    for c in range(chunks):
                        s0, s1 = c * cw, (c + 1) * cw
                        xt = pool.tile([_P, cw], f32, tag="x", bufs=xbufs,
                                       name=f"xt{rep}_{i}_{c}")
                        nc.sync.dma_start(out=xt[:], in_=x_d[r0:r1, s0:s1])
                        # d0 = (x <= 0.5)+0.5 -> {1.5 keep-always, 0.5 test-prev}
                        # On DVE: f32 tensor_scalar runs 2x_2P (2 elem/cyc);
                        # GPSIMD's fused-2-op tensor_scalar measured ~10x
                        # slower than modeled on real HW.
                        d0 = pool.tile([_P, cw], f32, tag="d", bufs=dbufs,
                                       name=f"d{rep}_{i}_{c}")
                        nc.vector.tensor_scalar(
                            out=d0[:], in0=xt[:], scalar1=0.5, scalar2=0.5,
                            op0=Alu.is_le, op1=Alu.add,
                        )
                        # out[t] = (d0[t] >= out[t-1]) * x[t]; in place over d0.
                        # Carry across chunks: initial = prev chunk's last col.
                        init = 0.0 if prev is None else prev[:, cw - 1:cw]
                        nc.vector.tensor_tensor_scan(
                            out=d0[:], data0=d0[:], data1=xt[:], initial=init,
                            op0=Alu.is_ge, op1=Alu.mult,
                        )
                        nc.scalar.dma_start(out=y_d[r0:r1, s0:s1], in_=d0[:])
                        prev = d0

    nc.compile()
    return nc


def _get_nc():
    if "nc" not in _cache:
        _cache["nc"] = _build()
    return _cache["nc"]


def _run(x, trace=False):
    from concourse.bass_utils import run_bass_kernel_spmd

    nc = _get_nc()
    x = np.ascontiguousarray(np.asarray(x, dtype=np.float32))
    assert x.shape == (_B, _S), x.shape
    in_maps = [
        {"x": np.ascontiguousarray(x[k * _RPC:(k + 1) * _RPC])} for k in range(_NC)
    ]
    res = run_bass_kernel_spmd(nc, in_maps, list(range(_NC)), trace=trace)
    out = np.concatenate([res.results[k]["y"] for k in range(_NC)], axis=0)
    return out, res


def kernel(x):
    out, _ = _run(x, trace=False)
    return out
